# revision 106
# baseline (speedup 1.0000x reference)
"""DGCNN edge-conv graph-feature module on Trainium2 (Bass/Tile).

Problem: for each batch (B=8): F-space KNN (k=20) over N=4096 points (C=64),
gather neighbor features, edge-MLP (128->128->64->64 with relu), max-pool
over the 20 neighbors -> (4096, 64).

Sharding: data-parallel over batch across the 8 NeuronCores (one batch each,
SPMD single NEFF).

Per-core algorithm:
  setup:
    xT (64x4096) and xT2 = 2*xT in float32r (full-rate fp32 PE path),
    -|x_j|^2 aug row, padded fp16 copy of x for gathering,
    v_i = (W1c-W1e) x_i + b1 per point (fp16), transposed weights
    (W3 block-diagonal [128,128] for partition-packed L3).
  per point-tile T (128 query rows):
    dist vals: val[i,j] = 2 x_i.x_j - |x_j|^2 via one f32r matmul per
      512-candidate block (augmented contraction row carries the -|x_j|^2).
    topk: per 512-block top-8 values + indices (DVE max8/max_index from
      PSUM, 3 psum bufs), then top-20-of-64 merge on DVE; per-row index
      compaction via cumsum + GPSIMD local_scatter.
    gather: per-group fold-DMA rewraps the 20 indices/row into the
      16-partition-wrapped order, then SBUF-source dma_gather (transposed)
      pulls the 20 neighbor features/row as (128ch x 2560 pairs) fp16.
    MLP: L1 = relu(W1e x_j + v_i) with the v_i term applied by a matmul
      against a k-repeated identity; L2 partition-packs two 64-ch chunks
      into one [128,320] psum (tile_position col-group) and L3 uses the
      block-diag W3 so one matmul + one relu evacuation covers both;
      max-pool over k on DVE (4-dim strided views); pooled fp16 PE
      transpose; nested-AP DMA writes rows back in point order.
  schedule: software pipeline with 2-group lookahead — group g+1's topk
    units (dist matmul + DVE scans) are woven instruction-by-instruction
    with group g-1's MLP units so the DVE topk stream never starves while
    ACT drains relu evacuations; fold+gather issue at the end of each
    woven iteration.
"""

import os
import sys
from itertools import zip_longest

for _p in ("/opt/trn_rl_repo", "/root/.axon_site/_ro/trn_rl_repo"):
    if os.path.isdir(_p) and _p not in sys.path:
        sys.path.insert(0, _p)

import numpy as np

import concourse.bass as bass
import concourse.mybir as mybir
from concourse import bacc
from concourse.bass_utils import run_bass_kernel_spmd
from concourse.masks import make_identity
from concourse.tile import TileContext

f32 = mybir.dt.float32
f32r = mybir.dt.float32r
f16 = mybir.dt.float16
i16 = mybir.dt.int16
u16 = mybir.dt.uint16

B, N, C, K = 8, 4096, 64, 20
C1, C2, C3 = 128, 64, 64
NT = N // 128              # point tiles per core
NBLK = N // 512            # candidate blocks per tile (topk block size 512)
NCAND = NBLK * 8           # merge candidates per row
PAIRS = 128 * K            # pairs per point tile (2560)
GROUP = 3                  # tiles per scatter/gather/pool phase group
NEG = -1e30


def build_nc(nt=NT, stage=None, repeat=1):
    if stage is None:
        stage = int(os.environ.get("KM_STAGE", "9"))
    nc = bacc.Bacc(None, target_bir_lowering=False)

    pts = nc.declare_dram_parameter("points", [N, C], f32, isOutput=False)
    w1 = nc.declare_dram_parameter("W1", [C1, 2 * C], f32, isOutput=False)
    b1 = nc.declare_dram_parameter("b1", [C1], f32, isOutput=False)
    w2 = nc.declare_dram_parameter("W2", [C2, C1], f32, isOutput=False)
    b2 = nc.declare_dram_parameter("b2", [C2], f32, isOutput=False)
    w3 = nc.declare_dram_parameter("W3", [C3, C2], f32, isOutput=False)
    b3 = nc.declare_dram_parameter("b3", [C3], f32, isOutput=False)
    out = nc.declare_dram_parameter("out", [N, C3], f32, isOutput=True)

    pool_chain = []  # GPSIMD extended-ISA ops, chained to batch ucode libraries

    with TileContext(nc) as tc:
        with tc.tile_pool(name="const", bufs=1) as cp:
            ident = cp.tile([128, 128], f32)
            make_identity(nc, ident)
            ident16 = cp.tile([128, 128], f16)
            nc.vector.tensor_copy(ident16, ident)

            # ---- load points as [p, T, c]; split so transposes start early
            x_sb = cp.tile([128, NT, C], f32)
            for q in range(4):
                t0, t1 = q * (NT // 4), (q + 1) * (NT // 4)
                src = bass.AP(
                    tensor=pts.ap().tensor, offset=t0 * 128 * C,
                    ap=[[C, 128], [128 * C, t1 - t0], [1, C]],
                )
                nc.sync.dma_start(out=x_sb[:, t0:t1, :], in_=src)

            # ---- weights + biases
            w1_sb = cp.tile([C1, 2 * C], f32)
            nc.sync.dma_start(out=w1_sb, in_=w1[:, :])
            w2_sb = cp.tile([C2, C1], f32)
            nc.sync.dma_start(out=w2_sb, in_=w2[:, :])
            w3_sb = cp.tile([C3, C2], f32)
            nc.sync.dma_start(out=w3_sb, in_=w3[:, :])
            b1_row = cp.tile([1, C1], f32)
            nc.sync.dma_start(out=b1_row, in_=b1.ap().rearrange("(a c) -> a c", a=1))
            # partition-packed biases: [b2; b2] and [b3; b3] on 128 partitions
            b2_col = cp.tile([128, 1], f32)
            nc.sync.dma_start(out=b2_col[0:C2, :],
                              in_=b2.ap().rearrange("(c a) -> c a", a=1))
            nc.sync.dma_start(out=b2_col[C2:2 * C2, :],
                              in_=b2.ap().rearrange("(c a) -> c a", a=1))
            b3_col = cp.tile([128, 1], f32)
            nc.sync.dma_start(out=b3_col[0:C3, :],
                              in_=b3.ap().rearrange("(c a) -> c a", a=1))
            nc.sync.dma_start(out=b3_col[C3:2 * C3, :],
                              in_=b3.ap().rearrange("(c a) -> c a", a=1))

            # slot -> 512*block offset constant for globalizing block-local idx
            offs = cp.tile([128, NCAND], u16)
            nc.gpsimd.iota(offs, pattern=[[512, NBLK], [0, 8]], base=0,
                           channel_multiplier=0)

            # ---- xTaug [65, N]: rows 0:64 = xT, row 64 = ones (lhsT + v lhsT)
            # ---- xT2aug [65, N]: rows 0:64 = 2*xT, row 64 = -|x_j|^2 (dist rhs)
            # f32r so the dist matmul runs the full-rate fp32 path
            xTaug = cp.tile([C + 1, N], f32r)
            xT2aug = cp.tile([C + 1, N], f32r)
            ones_row = cp.tile([1, N], f32)
            nc.vector.memset(ones_row, 1.0)
            nc.vector.tensor_copy(xTaug[C:C + 1, :], ones_row)
            x_f16 = cp.tile([128, NT, 128], f16)
            nc.vector.memset(x_f16, 0.0)
            v_all = cp.tile([128, NT, C1], f16)

            with tc.tile_pool(name="setup_ps", bufs=2, space="PSUM") as sps:
                # transpose W1, W2, W3
                w1t = cp.tile([2 * C, C1], f32)    # [c_in, c_out]
                p = sps.tile([128, 128], f32, tag="tp")
                nc.tensor.transpose(p, w1_sb, ident)
                nc.vector.tensor_copy(w1t, p)
                w1t16 = cp.tile([2 * C, C1], f16)
                nc.vector.tensor_copy(w1t16, w1t)

                w2t16 = cp.tile([C1, C2], f16)
                p = sps.tile([128, 128], f32, tag="tp")
                nc.tensor.transpose(p[:, 0:C2], w2_sb, ident[0:C2, 0:C2])
                nc.vector.tensor_copy(w2t16, p[0:C1, 0:C2])

                # w3 as block-diag [128, 128]: one matmul handles two
                # partition-packed 64-channel chunks
                w3blk = cp.tile([128, 128], f16)
                nc.vector.memset(w3blk, 0.0)
                p = sps.tile([128, 128], f32, tag="tp")
                nc.tensor.transpose(p[0:C2, 0:C3], w3_sb[:, :], ident[0:C2, 0:C2])
                nc.vector.tensor_copy(w3blk[0:C2, 0:C3], p[0:C2, 0:C3])
                nc.vector.tensor_copy(w3blk[C2:2 * C2, C3:2 * C3],
                                      p[0:C2, 0:C3])

                # Wv_aug [65, C1]: rows 0:64 = (W1c - W1e)^T, row 64 = b1
                wv = cp.tile([C + 1, C1], f32)
                delta = cp.tile([C1, C], f32)
                nc.vector.tensor_tensor(
                    out=delta, in0=w1_sb[:, C:2 * C], in1=w1_sb[:, 0:C],
                    op=mybir.AluOpType.subtract)
                p = sps.tile([128, 128], f32, tag="tp")
                nc.tensor.transpose(p[0:C, :], delta, ident)
                nc.vector.tensor_copy(wv[0:C, :], p[0:C, :])
                nc.sync.dma_start(out=wv[C:C + 1, :],
                                  in_=b1.ap().rearrange("(a c) -> a c", a=1))

                # per point-tile: transpose x, fill xTaug/xT2aug/x_f16
                # (evacuations split ACT/DVE to shorten the serial setup;
                # xsq squared per-quarter so the -|x|^2 row pipeline starts
                # before all transposes finish)
                xsq = cp.tile([C, N], f32)
                for t in range(NT):
                    p = sps.tile([128, 128], f32, tag="tp")
                    nc.tensor.transpose(p[0:C, :], x_sb[:, t, :], ident)
                    sl = slice(t * 128, (t + 1) * 128)
                    nc.vector.tensor_copy(xTaug[0:C, sl], p[0:C, :])
                    nc.scalar.activation(
                        xT2aug[0:C, sl], p[0:C, :],
                        mybir.ActivationFunctionType.Copy, scale=2.0)
                    nc.vector.tensor_copy(x_f16[:, t, 0:C], x_sb[:, t, :])
                    if t % 8 == 7:
                        qsl = slice((t - 7) * 128, (t + 1) * 128)
                        nc.scalar.activation(
                            xsq[:, qsl], xTaug[0:C, qsl].bitcast(f32),
                            mybir.ActivationFunctionType.Square)

                # -|x_j|^2 row: columns of xsq summed by ones-matmul
                negones = cp.tile([C, 1], f32)
                nc.vector.memset(negones, -1.0)
                for b in range(NBLK):
                    p = sps.tile([1, 512], f32, tag="sq")
                    nc.tensor.matmul(p, negones, xsq[:, b * 512:(b + 1) * 512],
                                     start=True, stop=True)
                    nc.scalar.activation(
                        xT2aug[C:C + 1, b * 512:(b + 1) * 512], p,
                        mybir.ActivationFunctionType.Copy)

                # v tiles: (x W_v + b1) per point, fp16, [i, T, ch]
                for t in range(NT):
                    p = sps.tile([128, 128], f32, tag="tp")
                    nc.tensor.matmul(
                        p, xTaug[:, t * 128:(t + 1) * 128].bitcast(f32),
                        wv, start=True, stop=True)
                    nc.vector.tensor_copy(v_all[:, t, :], p)

            # ---- per-tile index list + wrapped gather lists
            ilist = cp.tile([128, nt, 32], i16)
            wrap = cp.tile([128, nt, 160], i16)

            with tc.tile_pool(name="topk", bufs=8) as tk, \
                 tc.tile_pool(name="mlp", bufs=4) as mp, \
                 tc.tile_pool(name="xgp", bufs=8) as xgpool, \
                 tc.tile_pool(name="h2p", bufs=3) as h2p, \
                 tc.tile_pool(name="h3p", bufs=5) as h3p, \
                 tc.tile_pool(name="outp", bufs=3) as op_, \
                 tc.tile_pool(name="ps_dist", bufs=3, space="PSUM") as psd, \
                 tc.tile_pool(name="ps_l1", bufs=2, space="PSUM") as ps1, \
                 tc.tile_pool(name="ps_l2", bufs=1, space="PSUM") as ps2, \
                 tc.tile_pool(name="ps_l3", bufs=1, space="PSUM") as ps3, \
                 tc.tile_pool(name="ps_ot", bufs=1, space="PSUM") as pso:

                def topk_units(t):
                    # list of emission units: 8 per-block (mm+max8+maxidx)
                    # + 1 merge/compact; woven against mlp units so PE
                    # always has dist work queued between MLP batches
                    st = {}

                    def u_block(b):
                        def f():
                            if b == 0:
                                st["m"] = tk.tile([128, NCAND], f32,
                                                  tag="m_sb", name="m_sb")
                                st["l"] = tk.tile([128, NCAND], u16,
                                                  tag="lidx", name="lidx")
                            m_sb, lidx = st["m"], st["l"]
                            pd = psd.tile([128, 512], f32, tag="dist")
                            # f32r: full-rate fp32 matmul (>=256 moving cols)
                            nc.tensor.matmul(
                                pd, xTaug[:, t * 128:(t + 1) * 128],
                                xT2aug[:, b * 512:(b + 1) * 512],
                                start=True, stop=True)
                            nc.vector.max(out=m_sb[:, b * 8:(b + 1) * 8],
                                          in_=pd)
                            nc.vector.max_index(
                                out=lidx[:, b * 8:(b + 1) * 8],
                                in_max=m_sb[:, b * 8:(b + 1) * 8],
                                in_values=pd)
                        return f

                    units = [u_block(b) for b in range(NBLK)]
                    if stage >= 3:
                        units.append(lambda: topk_merge(t, st["m"], st["l"]))
                    return units

                def topk_tile(t):
                    for u in topk_units(t):
                        u()

                def topk_merge(t, m_sb, lidx):
                    # merge: top-20 of 64 -> per-row compacted gather list
                    work = tk.tile([128, NCAND], f32, tag="work")
                    t8 = tk.tile([128, 8], f32, tag="t8")
                    nc.vector.max(out=t8, in_=m_sb)
                    nc.vector.match_replace(
                        out=work, in_to_replace=t8, in_values=m_sb,
                        imm_value=NEG)
                    nc.vector.max(out=t8, in_=work)
                    nc.vector.match_replace(
                        out=work, in_to_replace=t8, in_values=work,
                        imm_value=NEG)
                    nc.vector.max(out=t8, in_=work)
                    qual = tk.tile([128, NCAND], f32, tag="qual")
                    # qual = m_sb >= v20 (v20 = 4th value of 3rd octet);
                    # rest of the compaction chain runs on gpsimd
                    nc.vector.tensor_scalar(
                        qual, m_sb, t8[:, 3:4], scalar2=None,
                        op0=mybir.AluOpType.is_ge)
                    pos = tk.tile([128, NCAND], f32, tag="pos")
                    nc.vector.tensor_tensor_scan(
                        out=pos, data0=qual, data1=qual, initial=0.0,
                        op0=mybir.AluOpType.add, op1=mybir.AluOpType.bypass)
                    # position = qual*cumsum - 1 (invalid -> -1), clamp <= 31
                    nc.gpsimd.tensor_mul(pos, pos, qual)
                    nc.vector.tensor_scalar(
                        pos, pos, 1.0, scalar2=31.0,
                        op0=mybir.AluOpType.subtract, op1=mybir.AluOpType.min)
                    pidx = tk.tile([128, NCAND], i16, tag="pidx")
                    nc.gpsimd.tensor_copy(pidx, pos)
                    gidx = tk.tile([128, NCAND], u16, tag="gidx")
                    nc.vector.tensor_tensor(
                        out=gidx, in0=lidx, in1=offs,
                        op=mybir.AluOpType.add)
                    sc = nc.gpsimd.local_scatter(
                        out_ap=ilist[:, t, :], data_ap=gidx.bitcast(i16),
                        idxs_ap=pidx, channels=128, num_elems=32,
                        num_idxs=NCAND)
                    pool_chain.append(sc.ins)

                def fold_group(g, tiles):
                    # rewrap idx lists: wrap[q, T, h*20+k] = ilist[16h+q, T, k]
                    for h in range(8):
                        nc.sync.dma_start(
                            out=wrap[0:16, tiles[0]:tiles[-1] + 1,
                                     h * 20:(h + 1) * 20],
                            in_=ilist[16 * h:16 * (h + 1),
                                      tiles[0]:tiles[-1] + 1, 0:20])
                    for g2 in range(1, 8):
                        nc.sync.dma_start(
                            out=wrap[16 * g2:16 * (g2 + 1),
                                     tiles[0]:tiles[-1] + 1, :],
                            in_=wrap[0:16, tiles[0]:tiles[-1] + 1, :])

                def gather_tile(t):
                    xg = xgpool.tile([128, 1, PAIRS], f16, tag="xg")
                    gi = nc.gpsimd.dma_gather(
                        out_ap=xg,
                        in_ap=x_f16.rearrange("p t c -> p (t c)"),
                        idxs_ap=wrap[:, t, :],
                        num_idxs=PAIRS,
                        num_idxs_reg=PAIRS,
                        elem_size=128,
                        transpose=True,
                        sbuf_tokens_per_rank=128,
                        sbuf_free_dim_per_rank=256,
                        sbuf_free_dim_pad_per_rank=0,
                        sbuf_byte_offset=0,
                        single_packet=False,
                    )
                    pool_chain.append(gi.ins)
                    return xg

                def mlp_units(t, xg):
                    # pair-granular units: each allocates and fully retires
                    # its own ps2 buffer, so units from different tiles can
                    # interleave without psum-pool deadlock
                    st = {}

                    def u_pair(cp_):
                        def f():
                            if cp_ == 0:
                                st["h3"] = h3p.tile([128, 4, K, 16], f16,
                                                    tag="h3", name="h3")
                            mlp_chunk(t, xg, 2 * cp_, st)
                            mlp_chunk(t, xg, 2 * cp_ + 1, st)
                        return f

                    return ([u_pair(cp_) for cp_ in range(4)]
                            + [lambda: pool_out_tile(t, st["h3"])])

                def mlp_chunk(t, xg, c, st):
                    # partition-packed L2/L3: chunk pair (2c, 2c+1) shares one
                    # [128, 320] psum (parities on partition halves)
                    h3 = st["h3"]
                    sl = slice(c * 320, (c + 1) * 320)
                    p1 = ps1.tile([C1, 320], f32, tag="l1")
                    rep = bass.AP(
                        tensor=ident16.tensor,
                        offset=ident16.offset + 16 * c * ident16.ap[-1][0],
                        ap=[ident16.ap[0], [0, K], [ident16.ap[-1][0], 16]],
                    )
                    nc.tensor.matmul(p1, v_all[:, t, :], rep,
                                     start=True, stop=False)
                    nc.tensor.matmul(p1, w1t16, xg[:, 0, sl],
                                     start=False, stop=True)
                    h1 = mp.tile([C1, 320], f16, tag="h1")
                    nc.scalar.activation(
                        h1, p1, mybir.ActivationFunctionType.Relu)
                    if c % 2 == 0:
                        st["p2"] = ps2.tile([128, 320], f32, tag="l2", name="p2")
                        nc.tensor.matmul(st["p2"][0:C2, :], w2t16, h1,
                                         start=True, stop=True)
                    else:
                        p2 = st["p2"]
                        nc.tensor.matmul(p2[C2:2 * C2, :], w2t16, h1,
                                         start=True, stop=True,
                                         tile_position=(0, 64))
                        h2 = h2p.tile([128, 320], f16, tag="h2")
                        nc.scalar.activation(
                            h2, p2, mybir.ActivationFunctionType.Relu,
                            bias=b2_col)
                        p3 = ps3.tile([128, 320], f32, tag="l3")
                        nc.tensor.matmul(p3, w3blk, h2,
                                         start=True, stop=True)
                        nc.scalar.activation(
                            h3[:, c // 2].rearrange("p a b -> p (a b)"),
                            p3, mybir.ActivationFunctionType.Relu,
                            bias=b3_col)

                def pool_out_tile(t, h3):
                    # h3: [128p=(parity, ch), 4 slot, 20 k, 16 q];
                    # point = 32*slot + 16*parity + q. Max over k: big levels
                    # on gpsimd, tail on DVE.
                    m10 = h3p.tile([128, 4, 10, 16], f16, tag="m10")
                    nc.vector.tensor_max(m10, h3[:, :, 0:10], h3[:, :, 10:20])
                    m5 = h3p.tile([128, 4, 5, 16], f16, tag="m5")
                    nc.vector.tensor_max(m5, m10[:, :, 0:5], m10[:, :, 5:10])
                    m2 = h3p.tile([128, 4, 2, 16], f16, tag="m2")
                    nc.vector.tensor_max(m2, m5[:, :, 0:2], m5[:, :, 2:4])
                    m1 = h3p.tile([128, 4, 1, 16], f16, tag="m1")
                    nc.vector.tensor_max(m1, m2[:, :, 0:1], m2[:, :, 1:2])
                    pooled = op_.tile([128, 4, 1, 16], f16, tag="pooled")
                    nc.vector.tensor_max(pooled, m1, m5[:, :, 4:5])
                    po = pso.tile([C3, 128], f16, tag="ot")
                    nc.tensor.transpose(
                        po, pooled.rearrange("p a b c -> p (a b c)"), ident16)
                    osb = op_.tile([C3, 128], f32, tag="osb")
                    nc.scalar.activation(osb, po,
                                         mybir.ActivationFunctionType.Copy)
                    # osb[16s+q, 64b+ch] -> out[t*128 + 32s+16b+q, ch]
                    for bh in range(2):
                        dst = bass.AP(
                            tensor=out.ap().tensor,
                            offset=(t * 128 + 16 * bh) * C3,
                            ap=[[32 * C3, 4], [C3, 16], [1, C3]],
                        )
                        nc.sync.dma_start(
                            out=dst, in_=osb[:, bh * C3:(bh + 1) * C3])

                # group sizes: small edge groups shrink pipeline fill/drain
                if nt == NT:
                    sizes = [2, 2, 5, 5, 5, 5, 4] + [2, 2]
                else:
                    sizes = [min(GROUP, nt - s)
                             for s in range(0, nt, GROUP)]
                bounds = [0]
                for s in sizes:
                    bounds.append(bounds[-1] + s)
                n_groups = len(sizes)

                def group_tiles(g):
                    return list(range(bounds[g], bounds[g + 1]))

                # software pipeline, lookahead 2: iteration `it` emits group
                # it's topk (PE dist + DVE scans) interleaved with group
                # it-2's MLP (PE mm + ACT relu), so the topk stream runs two
                # groups ahead and fold/gather latency never gates MLP.
                for _rep in range(repeat):
                    xgs = {}

                    def do_topk_group(g):
                        for tn in group_tiles(g):
                            topk_tile(tn)
                        finish_topk_group(g)

                    def finish_topk_group(g):
                        if stage >= 4:
                            fold_group(g, group_tiles(g))
                        if stage >= 5:
                            for tn in group_tiles(g):
                                xgs[tn] = gather_tile(tn)

                    do_topk_group(0)
                    for it in range(1, n_groups + 2):
                        tk_g, ml_g = it, it - 2
                        nxt = group_tiles(tk_g) if tk_g < n_groups else []
                        cur = (group_tiles(ml_g) if 0 <= ml_g < n_groups
                               else [])
                        if stage < 6 or not cur:
                            if nxt:
                                do_topk_group(tk_g)
                            continue
                        # weave the two unit streams proportionally so PE
                        # keeps dist matmuls queued between MLP batches
                        a = [u for tn in nxt for u in topk_units(tn)]
                        if nxt:
                            a.append(lambda: finish_topk_group(tk_g))
                        b_ = [u for t in cur for u in mlp_units(t, xgs.pop(t))]
                        na, nb = len(a), len(b_)
                        ia = ib = 0
                        while ia < na or ib < nb:
                            if (ib >= nb
                                    or (ia < na and ia * nb < ib * na)):
                                a[ia]()
                                ia += 1
                            else:
                                b_[ib]()
                                ib += 1

        # order GPSIMD extended-ISA ops to batch ucode-library reloads
        if os.environ.get("KM_CHAIN", "1") == "1":
            from concourse.tile_rust import add_dep_helper
            for a, b_ in zip(pool_chain, pool_chain[1:]):
                add_dep_helper(b_, a, sync=False, reason="gpsimd library batching")

    return nc


_nc_cache = None
_fn_cache = None
LAST_EXEC_NS = None


def _build_cached_fn(nc):
    """One persistent jitted SPMD callable (run_bass_kernel_spmd re-jits
    per call); mirrors bass2jax.run_bass_via_pjrt's plumbing."""
    import jax
    from jax.sharding import Mesh, PartitionSpec
    from jax.experimental.shard_map import shard_map
    from concourse.bass2jax import (
        _bass_exec_p, partition_id_tensor, install_neuronx_cc_hook)

    install_neuronx_cc_hook()
    pname = nc.partition_id_tensor.name if nc.partition_id_tensor else None
    in_names, out_names, out_avals, zero_outs = [], [], [], []
    for alloc in nc.m.functions[0].allocations:
        if not isinstance(alloc, mybir.MemoryLocationSet):
            continue
        name = alloc.memorylocations[0].name
        if alloc.kind == "ExternalInput":
            if name != pname:
                in_names.append(name)
        elif alloc.kind == "ExternalOutput":
            out_names.append(name)
            shape = tuple(alloc.tensor_shape)
            dtype = mybir.dt.np(alloc.dtype)
            out_avals.append(jax.core.ShapedArray(shape, dtype))
            zero_outs.append(np.zeros(shape, dtype))
    n_params, n_outs = len(in_names), len(out_avals)
    all_in = list(in_names) + out_names + ([pname] if pname else [])

    def _body(*args):
        operands = list(args)
        if pname is not None:
            operands.append(partition_id_tensor())
        return tuple(_bass_exec_p.bind(
            *operands, out_avals=tuple(out_avals), in_names=tuple(all_in),
            out_names=tuple(out_names), lowering_input_output_aliases=(),
            sim_require_finite=True, sim_require_nnan=True, nc=nc))

    devices = jax.devices()[:B]
    mesh = Mesh(np.asarray(devices), ("core",))
    fn = jax.jit(
        shard_map(_body, mesh=mesh,
                  in_specs=(PartitionSpec("core"),) * (n_params + n_outs),
                  out_specs=(PartitionSpec("core"),) * n_outs,
                  check_rep=False),
        donate_argnums=tuple(range(n_params, n_params + n_outs)),
        keep_unused=True)
    return fn, in_names, zero_outs


def kernel(points, W1, b1, W2, b2, W3, b3):
    global _nc_cache, _fn_cache, LAST_EXEC_NS
    if _nc_cache is None:
        _nc_cache = build_nc()
        # Bacc defers register allocation to compile(); the PJRT path
        # serializes the module as-is, so finalize before running.
        _nc_cache.finalize()
    nc = _nc_cache
    common = {
        "W1": np.ascontiguousarray(W1, dtype=np.float32),
        "b1": np.ascontiguousarray(b1, dtype=np.float32),
        "W2": np.ascontiguousarray(W2, dtype=np.float32),
        "b2": np.ascontiguousarray(b2, dtype=np.float32),
        "W3": np.ascontiguousarray(W3, dtype=np.float32),
        "b3": np.ascontiguousarray(b3, dtype=np.float32),
    }
    in_maps = [
        dict(common, points=np.ascontiguousarray(points[b], dtype=np.float32))
        for b in range(B)
    ]
    if os.environ.get("KM_NO_FASTPATH", "0") != "1":
        try:
            import jax
            if _fn_cache is None:
                _fn_cache = _build_cached_fn(nc)
            fn, in_names, zero_outs = _fn_cache
            concat_in = [
                np.concatenate([np.asarray(in_maps[c][nm])
                                for c in range(B)], axis=0)
                for nm in in_names
            ]
            zs = [np.zeros((B * z.shape[0], *z.shape[1:]), z.dtype)
                  for z in zero_outs]
            out = fn(*concat_in, *zs)
            arr = np.asarray(out[0]).reshape(B, N, C3)
            LAST_EXEC_NS = None
            return np.stack([arr[b] for b in range(B)], axis=0)
        except Exception:
            _fn_cache = None  # fall back to the stock path
    trace = os.environ.get("BASS_TRACE", "0") == "1"
    res = run_bass_kernel_spmd(nc, in_maps, list(range(B)), trace=trace)
    LAST_EXEC_NS = res.exec_time_ns
    return np.stack([res.results[b]["out"] for b in range(B)], axis=0)


if __name__ == "__main__":
    pts = np.random.randn(B, N, C).astype(np.float32)
    W1_ = (np.random.randn(C1, 2 * C) * 0.05).astype(np.float32)
    W2_ = (np.random.randn(C2, C1) * 0.05).astype(np.float32)
    W3_ = (np.random.randn(C3, C2) * 0.05).astype(np.float32)
    z1, z2, z3 = (np.zeros(C1, np.float32), np.zeros(C2, np.float32),
                  np.zeros(C3, np.float32))
    o = kernel(pts, W1_, z1, W2_, z2, W3_, z3)
    print(o.shape, o.dtype)

